# revision 30
# baseline (speedup 1.0000x reference)
"""Trainium2 Bass kernel for nn_BinaryConnectNet (binary CNN, 8 NeuronCores).

v2 design:
 - batch-parallel convs (128 img/core, two 64-img halves A/B), fc1
   output-feature-sharded (128 features/core) with per-half async AllGathers
   of the activations (fp8) hidden under conv2-B / fc-A compute.
 - conv1: dense 3x3 (dw+pw fused) as K=82 matmul (81 = 9 taps x 3ch x 3
   bf16-splits of x, +1 bias row), rhs from a host-prebuilt replicated
   layout so each row-pair is ONE large DMA (alternating sync/scalar rings).
   2x2 maxpool fused over 4 quadrant PSUMs: ACT signs 2 (+-1 bf16), DVE
   bitwise-signs 2 (one-op (x&0x80000000)|0x3F800000 -> +-1 f32), GpSimd
   3-max tree -> +-1 fp8 h1.
 - conv2 all-fp8 (exact small ints): depthwise 3x3 as 4 DoubleRow
   pair-matmuls + 1 single over shifted windows of h1 (diagonal weights);
   dw bias added at the ACT PSUM->SBUF eviction; pointwise K=128 fp8;
   pool quadrants evicted as {1,0} = is_ge(psum, thr) on DVE (thr folds the
   conv2 biases exactly), OR-tree on GpSimd -> h2 in {1,0} fp8.
 - fc1: weights hi/lo fp16 (exact), c'-major DRAM layout, 4 big DMAs to
   SBUF; rhs = gathered {1,0} fp8; true preact = 2*psum - rowsum(W), so
   s1 = ACT Sign(scale=2, bias=-rowsum)  (exact correction).
 - fc2: fp16 hi/lo per-core partials on +-1 s1, summed on host.

Phases: P0[conv1-A; conv1-B interleaved with conv2-A] ccA(async)
        P1[conv2-B] ccB(async)+waitA P2[fc-A] waitB P3[fc-B].
"""

import os
import sys

for _p in ("/opt/trn_rl_repo",):
    if _p not in sys.path:
        sys.path.insert(0, _p)

import numpy as np
import ml_dtypes
from contextlib import ExitStack

import concourse.bass as bass
import concourse.bacc as bacc
import concourse.mybir as mybir
import concourse.tile as tile
from concourse.bass_utils import run_bass_kernel_spmd

F32 = mybir.dt.float32
BF16 = mybir.dt.bfloat16
FP16 = mybir.dt.float16
FP8 = mybir.dt.float8e4
U32 = mybir.dt.uint32
AF = mybir.ActivationFunctionType
ALU = mybir.AluOpType
DRMODE = mybir.MatmulPerfMode.DoubleRow

NCORES = 8
B = 128                   # images per core
HB = B // 2               # images per half = 64
H = 32
HP = 34                   # padded input row
ROWL = HB * HP            # 2176: flattened (b, w) for one padded row, half
X82_SLACK = 16
X82_ROW = HP * ROWL + X82_SLACK
NH = 16                   # pooled rows after pool1
P1 = 16                   # conv2 spatial
P1P = 18                  # padded
P2 = 8                    # pooled spatial after pool2
NF1 = 1024
FPC = NF1 // NCORES       # 128 fc1 features per core
KFC = 256 * P2 * P2       # 16384
CIMG = 8                  # conv2 chunk images
GSUB = 2                  # images per dw psum tile (512 cols, 1 bank)

# dw tap pairing for DoubleRow: stride-1 pairs only (hw ifmap pair
# adjacency) + singles
DW_PAIRS = [((0, 0), (0, 1)), ((1, 0), (1, 1)), ((2, 0), (2, 1))]
DW_SINGLES = [(0, 2), (1, 2), (2, 2)]
USE_DR = not os.environ.get("BCN_NO_DR")


def _bf16(a):
    return np.asarray(a, dtype=ml_dtypes.bfloat16)


def _fp8(a):
    return np.asarray(a, dtype=ml_dtypes.float8_e4m3fn)


def _host_prep(x, w1_dw, b1_dw, w1_pw, b1_pw, w2_dw, b2_dw, w2_pw, b2_pw,
               fc1_w, fc1_b, fc2_w, fc2_b):
    """Build per-core device input arrays (numpy only)."""
    sgn = np.sign
    x = np.asarray(x, np.float32)

    # triple bf16 split of x (exact to 2^-24)
    x0 = _bf16(x)
    r1 = x - x0.astype(np.float32)
    x1 = _bf16(r1)
    r2 = r1 - x1.astype(np.float32)
    x2 = _bf16(r2)
    splits = [x0, x1, x2]

    # padded per-(core, half, cs) rows: xp[core, half, cs, h, b, w] (+slack)
    n_flat = (HP + 2) * ROWL
    xp = np.zeros((NCORES, 2, 9, HP + 2, HB, HP), dtype=ml_dtypes.bfloat16)
    for s in range(3):
        xs = splits[s].reshape(NCORES, 2, HB, 3, H, H)
        for c in range(3):
            xp[:, :, 3 * c + s, 1:33, :, 1:33] = xs[:, :, :, c].transpose(
                0, 1, 3, 2, 4)
    xpf = np.zeros((NCORES, 2, 9, n_flat + 8), dtype=ml_dtypes.bfloat16)
    xpf[:, :, :, :n_flat] = xp.reshape(NCORES, 2, 9, n_flat)

    # x82: replicated-shifted layout [2, 82, X82_ROW]
    x82 = np.zeros((NCORES, 2, 82, X82_ROW), dtype=ml_dtypes.bfloat16)
    n_main = HP * ROWL
    for du in range(3):
        for dv in range(3):
            t = 3 * du + dv
            off = du * ROWL + dv
            x82[:, :, 9 * t:9 * t + 9, :n_main] = \
                xpf[:, :, :, off:off + n_main]
    x82[:, :, 81, :] = 1.0

    # conv1 fused weights: lhsT [82, 128]; row 81 = bias
    s1dw = sgn(np.asarray(w1_dw, np.float32))[:, 0]        # [3,3,3]
    s1pw = sgn(np.asarray(w1_pw, np.float32))[:, :, 0, 0]  # [128,3]
    w1t = np.zeros((82, 128), dtype=ml_dtypes.bfloat16)
    for du in range(3):
        for dv in range(3):
            for c in range(3):
                for s in range(3):
                    w1t[9 * (3 * du + dv) + 3 * c + s] = _bf16(
                        s1pw[:, c] * s1dw[c, du, dv])
    b1eff = (sgn(np.asarray(b1_pw, np.float32))
             + s1pw @ sgn(np.asarray(b1_dw, np.float32)))
    w1t[81] = _bf16(b1eff)

    # conv2 depthwise: DR pair weights [128, 4, 2, 128] + single [128, 128]
    s2dw = sgn(np.asarray(w2_dw, np.float32))[:, 0]        # [128,3,3]
    dwp = np.zeros((128, len(DW_PAIRS), 2, 128), np.float32)
    for i, (t0, t1) in enumerate(DW_PAIRS):
        np.fill_diagonal(dwp[:, i, 0, :], s2dw[:, t0[0], t0[1]])
        np.fill_diagonal(dwp[:, i, 1, :], s2dw[:, t1[0], t1[1]])
    dws = np.zeros((128, len(DW_SINGLES), 128), np.float32)
    for i, (du, dv) in enumerate(DW_SINGLES):
        np.fill_diagonal(dws[:, i, :], s2dw[:, du, dv])
    dwb = sgn(np.asarray(b2_dw, np.float32)).astype(np.float32)  # [128]

    # conv2 pointwise [128, 256] fp8; dw bias is added at dwc eviction so
    # the pw sign bias is just sgn(b2_pw)
    s2pw = sgn(np.asarray(w2_pw, np.float32))[:, :, 0, 0]  # [256,128]
    pwt = _fp8(s2pw.T)
    b2s = sgn(np.asarray(b2_pw, np.float32)).astype(np.float32)  # [256]

    # fc1 hi/lo fp16, c'-major: w[c', (ct*64+s0)*128 + o_local]
    fc1_w = np.asarray(fc1_w, np.float32).reshape(NCORES, FPC, KFC)
    # fc2 hi/lo per-core [128, 10]
    fc2_w = np.asarray(fc2_w, np.float32)                  # [10, 1024]
    f2 = fc2_w.T.reshape(NCORES, FPC, 10)
    f2hi = f2.astype(np.float16)
    f2lo = (f2 - f2hi.astype(np.float32)).astype(np.float16)

    per_core = []
    shared = {
        "w1t": w1t,
        "dwp": _fp8(dwp.reshape(128, -1)),
        "dws": _fp8(dws.reshape(128, -1)),
        "dwb": dwb.reshape(128, 1),
        "pwt": pwt,
        "b2s": b2s.reshape(2, 128).T.copy(),
    }
    for n in range(NCORES):
        wn = fc1_w[n]                                      # [128 o, 16384 k]
        hi = wn.astype(np.float16)
        lo = (wn - hi.astype(np.float32)).astype(np.float16)
        # [o, ct, c', s0] -> [c', ct, s0, o]
        whi = hi.reshape(FPC, 2, 128, 64).transpose(2, 1, 3, 0).reshape(
            128, KFC).copy()
        wlo = lo.reshape(FPC, 2, 128, 64).transpose(2, 1, 3, 0).reshape(
            128, KFC).copy()
        d = dict(shared)
        d["x82"] = x82[n]
        d["whi"] = whi
        d["wlo"] = wlo
        d["f2hi"] = f2hi[n]
        d["f2lo"] = f2lo[n]
        per_core.append(d)
    return per_core


def build_program(ncores=NCORES):
    nc = bacc.Bacc("TRN2", target_bir_lowering=False, debug=False,
                   num_devices=ncores)

    def din(name, shape, dt):
        return nc.dram_tensor(name, shape, dt, kind="ExternalInput").ap()

    x82 = din("x82", [2, 82, X82_ROW], BF16)
    w1t_d = din("w1t", [82, 128], BF16)
    dwp_d = din("dwp", [128, len(DW_PAIRS) * 256], FP8)
    dws_d = din("dws", [128, len(DW_SINGLES) * 128], FP8)
    dwb_d = din("dwb", [128, 1], F32)
    pwt_d = din("pwt", [128, 256], FP8)
    b2s_d = din("b2s", [128, 2], F32)
    whi_d = din("whi", [128, KFC], FP16)
    wlo_d = din("wlo", [128, KFC], FP16)
    f2hi_d = din("f2hi", [FPC, 10], FP16)
    f2lo_d = din("f2lo", [FPC, 10], FP16)
    nbh = ncores * HB              # fc column count per half
    y_out = nc.dram_tensor("y", [2, 10, nbh], F32,
                           kind="ExternalOutput").ap()

    # collective buffers: per half, [mt, 128, 64img*64pix] fp8
    h2_shard = nc.dram_tensor("h2_shard", [2, 2, 128, HB * 64], FP8).ap()
    h2_all = nc.dram_tensor("h2_all", [2, ncores, 2, 128, HB * 64], FP8,
                            addr_space="Shared").ap()

    dbg = os.environ.get("BCN_DEBUG")
    if dbg:
        dbg_h1 = nc.dram_tensor("dbg_h1", [2, 128, HB * P1P * P1P], FP8,
                                kind="ExternalOutput").ap()
        dbg_h2 = nc.dram_tensor("dbg_h2", [2, 2, 128, HB * 64], FP8,
                                kind="ExternalOutput").ap()
        dbg_dwc = nc.dram_tensor("dbg_dwc", [2, 8, 128, CIMG * 256], FP8,
                                 kind="ExternalOutput").ap()
        dbg_w = nc.dram_tensor("dbg_w", [128, 1536], FP8,
                               kind="ExternalOutput").ap()

    top = ExitStack()
    ccA_sem = top.enter_context(nc.semaphore("ccA_sem"))
    ccB_sem = top.enter_context(nc.semaphore("ccB_sem"))

    # persistent raw SBUF tensors (live across tile contexts)
    whi_sb = nc.alloc_sbuf_tensor("whi_sb", [128, KFC], FP16).ap()
    wlo_sb = nc.alloc_sbuf_tensor("wlo_sb", [128, KFC], FP16).ap()
    f2hi_t = nc.alloc_sbuf_tensor("f2hi_sb", [FPC, 10], FP16).ap()
    f2lo_t = nc.alloc_sbuf_tensor("f2lo_sb", [FPC, 10], FP16).ap()
    h1B_sb = nc.alloc_sbuf_tensor("h1B_sb", [128, HB * P1P * P1P], FP8).ap()
    h1vB = h1B_sb.rearrange("p (b y x) -> p b y x", b=HB, y=P1P)

    deferred = []

    def flush_deferred():
        for f in deferred:
            f()
        deferred.clear()

    def emit_conv1_step(pools, half, h1v, hc):
        imp, c1ps, spool, w1_t = (pools["imp"], pools["c1ps"],
                                  pools["spool"], pools["w1_t"])
        imt = imp.tile([82, 2 * ROWL], BF16, tag="imt",
                       name=f"imt{half}{hc}")
        nc.gpsimd.dma_start(imt[:], x82[half, :, 2 * hc * ROWL:
                                        (2 * hc + 2) * ROWL])
        imv = imt[:].rearrange("p (h b w2 dx) -> p h b w2 dx",
                               h=2, b=HB, w2=HP // 2)
        flush_deferred()
        # 32-image column halves j; per j: 4 quadrant psums [128, 512]
        # evicted as a running-max chain: ACT copy, 3x DVE stt-max, then a
        # DEFERRED ACT sign (emitted at the next step so the ACT FIFO's
        # sign(j) does not block the next chain's head copy)
        for j in range(2):
            prev = None
            for qi, (dy, dx) in enumerate(((0, 0), (0, 1),
                                           (1, 0), (1, 1))):
                ps = c1ps.tile([128, 512], F32, tag="c1ps", name="c1q")
                nc.tensor.matmul(
                    ps[:], w1_t[:],
                    imv[:, dy, j * 32:(j + 1) * 32, 0:16, dx],
                    start=True, stop=True)
                cur = spool.tile([128, 512], F32, tag="sm", name="cm")
                if qi == 0:
                    nc.scalar.copy(cur[:], ps[:])
                else:
                    nc.vector.scalar_tensor_tensor(
                        cur[:], ps[:], 0.0, prev[:], ALU.bypass, ALU.max)
                prev = cur
            def _sign(prev=prev, j=j, hc=hc, h1v=h1v):
                nc.scalar.activation(
                    h1v[:, j * 32:(j + 1) * 32, hc + 1, 1:17],
                    prev[:].rearrange("p (b x) -> p b x", b=32), AF.Sign)
            deferred.append(_sign)

    def emit_conv2_step(pools, half, h1v, h2t, g):
        dpool, dps, pps, upool = (pools["dpool"], pools["dps"],
                                  pools["pps"], pools["upool"])
        dwpt_t, dwst_t, dwb_t, pw_t, b2s_t = (
            pools["dwpt_t"], pools["dwst_t"], pools["dwb_t"],
            pools["pw_t"], pools["b2s_t"])
        b0 = g * CIMG
        flush_deferred()
        dwc = dpool.tile([128, CIMG * 256], FP8, tag="dwc", name="dwc")
        subs = [dps.tile([128, GSUB * 256], F32, tag="dps",
                         name=f"dsub{s}") for s in range(CIMG // GSUB)]
        # pass order: one single first (DR matmuls must not carry
        # start=True: with start, the two DR subtiles overwrite
        # per-element instead of accumulating)
        passes = ([("s", 0), ("p", 0), ("p", 1), ("p", 2),
                   ("s", 1), ("s", 2)] if USE_DR
                  else [("t", t) for t in range(9)])
        npass = len(passes)
        for t, (kind, ti) in enumerate(passes):
            first = t == 0
            last = t == npass - 1
            for si, ps in enumerate(subs):
                bb = b0 + si * GSUB
                if USE_DR:
                    if kind == "p":
                        (du0, dv0), (du1, dv1) = DW_PAIRS[ti]
                        for bi in range(GSUB):
                            base = h1v[:, bb + bi, du0:du0 + 16,
                                       dv0:dv0 + 16]
                            rhs = base.unsqueeze(1)
                            rhs.ap[1] = [(du1 - du0) * P1P
                                         + (dv1 - dv0), 2]
                            nc.tensor.matmul(
                                ps[:, bi * 256:(bi + 1) * 256],
                                dwpt_t[:, ti], rhs,
                                start=False, stop=last, perf_mode=DRMODE)
                    else:
                        du, dv = DW_SINGLES[ti]
                        nc.tensor.matmul(
                            ps[:], dwst_t[:, ti],
                            h1v[:, bb:bb + GSUB, du:du + 16, dv:dv + 16],
                            start=first, stop=last)
                else:
                    du, dv = ti // 3, ti % 3
                    pi = [p for p in range(3) if DW_PAIRS[p][0]
                          == (du, dv) or DW_PAIRS[p][1] == (du, dv)]
                    if pi:
                        k = 0 if DW_PAIRS[pi[0]][0] == (du, dv) else 1
                        w_ap = dwpt_t[:, pi[0], k]
                    else:
                        w_ap = dwst_t[:, DW_SINGLES.index((du, dv))]
                    nc.tensor.matmul(
                        ps[:], w_ap,
                        h1v[:, bb:bb + GSUB, du:du + 16, dv:dv + 16],
                        start=first, stop=last)
        for si, ps in enumerate(subs):
            nc.scalar.activation(
                dwc[:, si * GSUB * 256:(si + 1) * GSUB * 256], ps[:],
                AF.Identity, bias=dwb_t[:])
        if dbg:
            nc.sync.dma_start(dbg_dwc[half, g], dwc[:])
        dwv = dwc[:].rearrange("p (b y2 dy x2 dx) -> p b y2 dy x2 dx",
                               b=CIMG, y2=P2, dy=2, x2=P2)
        for mt in range(2):
            prev = None
            for qi, (dy, dx) in enumerate(((0, 0), (0, 1),
                                           (1, 0), (1, 1))):
                ps = pps.tile([128, CIMG * 64], F32, tag="pps", name="pwq")
                nc.tensor.matmul(ps[:], pw_t[:, mt * 128:(mt + 1) * 128],
                                 dwv[:, :, :, dy, :, dx],
                                 start=True, stop=True)
                cur = upool.tile([128, CIMG * 64], F32, tag="uq", name="um")
                if qi == 0:
                    nc.scalar.copy(cur[:], ps[:])
                else:
                    nc.vector.scalar_tensor_tensor(
                        cur[:], ps[:], 0.0, prev[:], ALU.bypass, ALU.max)
                prev = cur
            # h2 stored s-major: col = s*HB + b (contiguous b for fc rhs)
            h2s = h2t[mt][:].rearrange("p (s b) -> p b s", s=64)
            def _sign(prev=prev, h2s=h2s, b0=b0, mt=mt):
                nc.scalar.activation(
                    h2s[:, b0:b0 + CIMG, :],
                    prev[:].rearrange("p (b s) -> p b s", b=CIMG),
                    AF.Sign, bias=b2s_t[:, mt:mt + 1])
            deferred.append(_sign)

    def alloc_conv2_pools(tc, ctx, cp_consts):
        d = dict(cp_consts)
        d["dpool"] = ctx.enter_context(tc.tile_pool(name="dwc", bufs=2))
        d["dps"] = ctx.enter_context(tc.tile_pool(name="dps", bufs=4,
                                                  space="PSUM"))
        d["pps"] = ctx.enter_context(tc.tile_pool(name="pps", bufs=2,
                                                  space="PSUM"))
        d["upool"] = ctx.enter_context(tc.tile_pool(name="uq", bufs=6))
        return d

    def load_conv_consts(tc, ctx):
        cp = ctx.enter_context(tc.tile_pool(name="consts", bufs=1))
        w1_t = cp.tile([82, 128], BF16)
        nc.sync.dma_start(w1_t[:], w1t_d[:])
        dwpt_t = cp.tile([128, len(DW_PAIRS), 2, 128], FP8)
        nc.sync.dma_start(
            dwpt_t[:].rearrange("p a b c -> p (a b c)"), dwp_d[:])
        dwst_t = cp.tile([128, len(DW_SINGLES), 128], FP8)
        nc.sync.dma_start(
            dwst_t[:].rearrange("p a c -> p (a c)"), dws_d[:])
        dwb_t = cp.tile([128, 1], F32)
        nc.sync.dma_start(dwb_t[:], dwb_d[:])
        pw_t = cp.tile([128, 256], FP8)
        nc.sync.dma_start(pw_t[:], pwt_d[:])
        b2s_t = cp.tile([128, 2], F32)
        nc.sync.dma_start(b2s_t[:], b2s_d[:])
        return {"w1_t": w1_t, "dwpt_t": dwpt_t, "dwst_t": dwst_t,
                "dwb_t": dwb_t, "pw_t": pw_t, "b2s_t": b2s_t}

    # ============ P0: conv1-A; conv1-B || conv2-A; ship-A ============
    with tile.TileContext(nc) as tc, ExitStack() as ctx:
        consts = load_conv_consts(tc, ctx)
        h1pool = ctx.enter_context(tc.tile_pool(name="h1p", bufs=1))
        h1A = h1pool.tile([128, HB * P1P * P1P], FP8)
        h1vA = h1A[:].rearrange("p (b y x) -> p b y x", b=HB, y=P1P)
        for h1v in (h1vA, h1vB):
            nc.vector.memset(h1v[:, :, 0, :], 0.0)
            nc.vector.memset(h1v[:, :, P1P - 1, :], 0.0)
            nc.vector.memset(h1v[:, :, 1:P1P - 1, 0], 0.0)
            nc.vector.memset(h1v[:, :, 1:P1P - 1, P1P - 1], 0.0)

        # HAM warmup: ~5us of matmuls so conv1 starts at 2.4GHz
        with tc.tile_pool(name="warm", bufs=1) as wmp, \
             tc.tile_pool(name="warmps", bufs=1, space="PSUM") as wps:
            wz = wmp.tile([128, 512], BF16)
            nc.vector.memset(wz[:], 0.0)
            wp_t = wps.tile([128, 512], F32)
            for _w in range(24):
                nc.tensor.matmul(wp_t[:], wz[:, 0:128], wz[:],
                                 start=(_w == 0), stop=(_w == 23))

        h2pool = ctx.enter_context(tc.tile_pool(name="h2", bufs=1))
        h2tA = [h2pool.tile([128, HB * 64], FP8, tag=f"h2a{m}",
                            name=f"h2a{m}") for m in range(2)]

        pools = dict(consts)
        pools["imp"] = ctx.enter_context(tc.tile_pool(name="imp", bufs=3))
        pools["c1ps"] = ctx.enter_context(
            tc.tile_pool(name="c1ps", bufs=2, space="PSUM"))
        pools["spool"] = ctx.enter_context(tc.tile_pool(name="sq", bufs=6))
        pools = alloc_conv2_pools(tc, ctx, pools)

        for hc in range(NH):
            emit_conv1_step(pools, 0, h1vA, hc)
        # fc weights -> raw SBUF via scalar ring; transfers overlap conv
        nc.scalar.dma_start(whi_sb[:, 0:8192], whi_d[:, 0:8192])
        nc.scalar.dma_start(wlo_sb[:, 0:8192], wlo_d[:, 0:8192])
        nc.scalar.dma_start(whi_sb[:, 8192:], whi_d[:, 8192:])
        nc.scalar.dma_start(wlo_sb[:, 8192:], wlo_d[:, 8192:])
        nc.scalar.dma_start(f2hi_t, f2hi_d[:])
        nc.scalar.dma_start(f2lo_t, f2lo_d[:])
        for i in range(8):
            emit_conv2_step(pools, 0, h1vA, h2tA, i)
            emit_conv1_step(pools, 1, h1vB, 2 * i)
            emit_conv1_step(pools, 1, h1vB, 2 * i + 1)
        flush_deferred()
        for mt in range(2):
            nc.gpsimd.dma_start(h2_shard[0, mt], h2tA[mt][:])
        if dbg:
            nc.sync.dma_start(dbg_h1[0], h1A[:])
            nc.sync.dma_start(dbg_h1[1], h1B_sb)
            for mt in range(2):
                nc.sync.dma_start(dbg_h2[0, mt], h2tA[mt][:])

    # -------- issue ccA (async, no wait) --------
    if ncores > 1:
        with nc.Block() as blk:
            @blk.gpsimd
            def _(gp):
                gp.collective_compute(
                    "AllGather", ALU.bypass,
                    replica_groups=[list(range(ncores))],
                    ins=[h2_shard[0]], outs=[h2_all[0]],
                ).then_inc(ccA_sem)

    # ============ P1: conv2-B; ship-B ============
    with tile.TileContext(nc) as tc, ExitStack() as ctx:
        consts = load_conv_consts(tc, ctx)
        h2pool = ctx.enter_context(tc.tile_pool(name="h2b", bufs=1))
        h2tB = [h2pool.tile([128, HB * 64], FP8, tag=f"h2b{m}",
                            name=f"h2b{m}") for m in range(2)]
        pools = alloc_conv2_pools(tc, ctx, consts)
        for g in range(8):
            emit_conv2_step(pools, 1, h1vB, h2tB, g)
        flush_deferred()
        for mt in range(2):
            nc.gpsimd.dma_start(h2_shard[1, mt], h2tB[mt][:])
        if dbg:
            for mt in range(2):
                nc.sync.dma_start(dbg_h2[1, mt], h2tB[mt][:])

    # -------- issue ccB (async), wait ccA --------
    if ncores > 1:
        with nc.Block() as blk:
            @blk.gpsimd
            def _(gp):
                gp.collective_compute(
                    "AllGather", ALU.bypass,
                    replica_groups=[list(range(ncores))],
                    ins=[h2_shard[1]], outs=[h2_all[1]],
                ).then_inc(ccB_sem)
                gp.wait_ge(ccA_sem, 1)
        nc.all_engine_barrier()
    else:
        with nc.Block() as blk, nc.semaphore("cp_sem") as cp_sem:
            @blk.gpsimd
            def _(gp):
                gp.dma_start(h2_all[0, 0], h2_shard[0]).then_inc(cp_sem, 16)
                gp.dma_start(h2_all[1, 0], h2_shard[1]).then_inc(cp_sem, 16)
                gp.wait_ge(cp_sem, 32)
        nc.all_engine_barrier()

    # ============ P2/P3: fc (half A, wait ccB, half B) ============
    def emit_fc_half(nc, half, hgp, fps, p10, spool2):
        psf = fps.tile([128, nbh], F32, tag="psf", name="psf")
        hgt = []
        for ct in range(2):
            t = hgp.tile([128, ncores, HB * 64], FP8, tag=f"hg{ct}",
                         name=f"hg{ct}")
            src = h2_all[half, :, ct].rearrange("sh p f -> p sh f")
            nc.gpsimd.dma_start(t[:], src)
            # s-major: [p, sh, s, b]
            hgt.append(t[:].rearrange("p sh (s b) -> p sh s b", s=64))
        first = True
        for ct in range(2):
            for s0 in range(64):
                col = (ct * 64 + s0) * 128
                rhs = hgt[ct][:, :, s0, :]
                nc.tensor.matmul(psf[:], whi_sb[:, col:col + 128], rhs,
                                 start=first, stop=False)
                first = False
                nc.tensor.matmul(psf[:], wlo_sb[:, col:col + 128], rhs,
                                 start=False,
                                 stop=(ct == 1 and s0 == 63))
        s1 = spool2.tile([128, nbh], FP16, tag="s1", name="s1")
        nc.scalar.activation(s1[:], psf[:], AF.Sign)
        ps10 = p10.tile([10, nbh], F32, tag="ps10", name="ps10")
        nc.tensor.matmul(ps10[:], f2hi_t, s1[:], start=True, stop=False)
        nc.tensor.matmul(ps10[:], f2lo_t, s1[:], start=False, stop=True)
        yt = spool2.tile([10, nbh], F32, tag="yt", name="yt")
        nc.scalar.copy(yt[:], ps10[:])
        nc.sync.dma_start(y_out[half], yt[:])

    for half in range(2):
        with tile.TileContext(nc) as tc2, ExitStack() as ctx2:
            hgp = ctx2.enter_context(tc2.tile_pool(name="hg", bufs=1))
            fps = ctx2.enter_context(tc2.tile_pool(name="fps", bufs=2,
                                                   space="PSUM"))
            p10 = ctx2.enter_context(tc2.tile_pool(name="p10", bufs=2,
                                                   space="PSUM"))
            spool2 = ctx2.enter_context(tc2.tile_pool(name="sp", bufs=2))
            emit_fc_half(nc, half, hgp, fps, p10, spool2)
        if half == 0 and ncores > 1:
            with nc.Block() as blk:
                @blk.gpsimd
                def _(gp):
                    gp.wait_ge(ccB_sem, 1)
            nc.all_engine_barrier()

    nc.compile()
    top.close()
    return nc


_CACHE = {}


def _get_program(ncores=NCORES):
    if ncores not in _CACHE:
        _CACHE[ncores] = build_program(ncores)
    return _CACHE[ncores]


def _assemble(res, fc2_b, ncores=NCORES):
    y8 = np.zeros((2, 10, ncores * HB), np.float32)
    for n in range(ncores):
        y8 += res.results[n]["y"]
    # col = sh*64 + b ; img = sh*128 + half*64 + b
    y8 = y8.reshape(2, 10, ncores, HB)
    y = np.zeros((NCORES * B, 10), np.float32)
    for half in range(2):
        for sh in range(ncores):
            y[sh * B + half * HB:sh * B + (half + 1) * HB] = \
                y8[half, :, sh, :].T
    return (y + np.asarray(fc2_b, np.float32)[None, :]).astype(np.float32)


def kernel(**inputs):
    per_core = _host_prep(**inputs)
    nc = _get_program()
    res = run_bass_kernel_spmd(nc, per_core, core_ids=list(range(NCORES)))
    return _assemble(res, inputs["fc2_b"])
